# revision 5
# baseline (speedup 1.0000x reference)
"""Trainium2 Bass kernel for MoE top-2 routing (softmax + l_aux + combine weights).

Problem: logits/mask1/mask2 [8192, 64], locations1/2 one-hot [8192, 256].
Outputs: l_aux scalar and combine_weights [8192, 64, 256].

Key structural facts exploited:
  * mask1/mask2 are disjoint one-hot rows and locations are one-hot rows, so
    combine_weights has exactly 2 nonzero elements per token out of 64*256.
    Each nonzero "row" combine_weights[s, e_k, :] equals g_k[s] * loc_k[s, :].
    We only *scatter* 2 rows of 256 floats per token into the (pre-zeroed)
    output via indirect DMA instead of materializing 512 MiB. The Q7
    descriptor generation (~18 ns/desc) dominates, so the whole rest of the
    kernel is arranged to hide behind the 16 scatter instructions.
  * g1 = num1/(num1+num2) with num_k = exp(top-k logit) simplifies to
    g1 = sigmoid(max - secondmax), g2 = sigmoid(secondmax - max); the
    softmax denominator cancels and the eps clamp never binds.
  * l_aux only needs column sums of gates (softmax) and mask1; both are
    computed as recip/ones-weighted PE matmuls accumulated in PSUM, entirely
    off the critical path; each core emits partial sums and the host
    finishes the tiny reduction while unsharding.

Sharding: tokens split 8 ways (1024 tokens per core); no cross-core
communication needed on device. Per-core token s = 8*p + j lives on
partition p, column group j (contiguous DRAM rows per partition => large
DMA descriptors on loads).
"""

import numpy as np

import concourse.bass as bass
import concourse.bacc as bacc
import concourse.mybir as mybir
from concourse.tile import TileContext
from concourse import bass_utils

S, E, C = 8192, 64, 256
N_CORES = 8
S_LOC = S // N_CORES          # 1024 tokens per core
P = 128                       # partitions
J = S_LOC // P                # 8 tokens per partition
F32 = mybir.dt.float32
AX = mybir.AxisListType.X
MUL = mybir.AluOpType.mult
ADD = mybir.AluOpType.add
SUB = mybir.AluOpType.subtract
EXP = mybir.ActivationFunctionType.Exp
SIG = mybir.ActivationFunctionType.Sigmoid
COPY = mybir.ActivationFunctionType.Copy

_CACHE = {}


def _build():
    nc = bacc.Bacc("TRN2", target_bir_lowering=False,
                   dynamic_dma_scratch_size=65536)

    logits = nc.declare_dram_parameter("logits", [S_LOC, E], F32, isOutput=False)
    m1d = nc.declare_dram_parameter("mask1", [S_LOC, E], F32, isOutput=False)
    m2d = nc.declare_dram_parameter("mask2", [S_LOC, E], F32, isOutput=False)
    l1d = nc.declare_dram_parameter("loc1", [S_LOC, C], F32, isOutput=False)
    l2d = nc.declare_dram_parameter("loc2", [S_LOC, C], F32, isOutput=False)
    cw = nc.declare_dram_parameter("cw", [S_LOC * E, C], F32, isOutput=True)
    partials = nc.declare_dram_parameter("partials", [1, 2 * E], F32, isOutput=True)

    with TileContext(nc) as tc:
        with (
            tc.tile_pool(name="sbuf", bufs=1) as pool,
            tc.tile_pool(name="psum", bufs=1, space="PSUM") as psum_pool,
        ):
            def v3(tile, inner):  # [P, J*inner] -> [P, J, inner]
                return tile[:].rearrange("p (j i) -> p j i", j=J)

            # ---- input loads (HWDGE, contiguous rows per partition) ----
            m1 = pool.tile([P, J * E], F32)
            nc.sync.dma_start(m1[:], m1d[:].rearrange("(p j) e -> p (j e)", p=P))
            lt = pool.tile([P, J * E], F32)
            nc.sync.dma_start(lt[:], logits[:].rearrange("(p j) e -> p (j e)", p=P))
            m2 = pool.tile([P, J * E], F32)
            nc.sync.dma_start(m2[:], m2d[:].rearrange("(p j) e -> p (j e)", p=P))
            l1 = pool.tile([P, J * C], F32)
            l2 = pool.tile([P, J * C], F32)
            l1v = l1d[:].rearrange("(p j) c -> p (j c)", p=P)
            l2v = l2d[:].rearrange("(p j) c -> p (j c)", p=P)
            Q = 2 * C  # quarter of the row (2 tokens)
            for qi in range(4):
                nc.sync.dma_start(l1[:, qi * Q:(qi + 1) * Q], l1v[:, qi * Q:(qi + 1) * Q])
            for qi in range(4):
                nc.sync.dma_start(l2[:, qi * Q:(qi + 1) * Q], l2v[:, qi * Q:(qi + 1) * Q])

            # rowvals[p, j*E + e] = (8p+j)*E + e via iota (no DMA needed)
            rvi = pool.tile([P, J * E], mybir.dt.int32)
            nc.gpsimd.iota(rvi[:], pattern=[[E, J], [1, E]], base=0,
                           channel_multiplier=J * E)
            rv = pool.tile([P, J * E], F32)
            nc.vector.tensor_copy(rv[:], rvi[:])

            # ---- scatter row indices (critical: term1 first) ----
            ridx = []
            for k, mk in enumerate((m1, m2)):
                q = pool.tile([P, J * E], F32, tag=f"q{k}")
                nc.vector.tensor_tensor(v3(q, E), v3(mk, E), v3(rv, E), op=MUL)
                rf = pool.tile([P, J], F32, tag=f"rf{k}")
                nc.vector.reduce_sum(rf[:], v3(q, E), axis=AX)
                ri = pool.tile([P, J], mybir.dt.int32, tag=f"ri{k}")
                nc.vector.tensor_copy(ri[:], rf[:])
                ridx.append(ri)

            # ---- gate values: g1 = sigmoid(max - secondmax), g2 = 1 - g1 ----
            rmax = pool.tile([P, J], F32)
            nc.vector.reduce_max(rmax[:], v3(lt, E), axis=AX)
            p2 = pool.tile([P, J * E], F32)
            nc.vector.tensor_tensor(v3(p2, E), v3(m2, E), v3(lt, E), op=MUL)
            sm2 = pool.tile([P, J], F32)  # secondmax (mask2 selects it; rest<=0 adds 0... not true, logits can be negative)
            nc.vector.reduce_sum(sm2[:], v3(p2, E), axis=AX)
            dlt = pool.tile([P, J], F32)
            nc.vector.tensor_tensor(dlt[:], rmax[:], sm2[:], op=SUB)
            g1 = pool.tile([P, J], F32)
            nc.scalar.activation(g1[:], dlt[:], SIG)
            ndlt = pool.tile([P, J], F32)
            nc.vector.tensor_scalar_mul(ndlt[:], dlt[:], -1.0)
            g2 = pool.tile([P, J], F32)
            nc.scalar.activation(g2[:], ndlt[:], SIG)

            # ---- payload rows + scatters, quarter by quarter ----
            r1 = pool.tile([P, J * C], F32)
            r2 = pool.tile([P, J * C], F32)
            g1b = g1[:].broadcast_to([P, J, C])
            for qi in range(4):
                js = slice(2 * qi, 2 * qi + 2)
                nc.vector.tensor_tensor(v3(r1, C)[:, js], v3(l1, C)[:, js],
                                        g1b[:, js], op=MUL)
                for j in (2 * qi, 2 * qi + 1):
                    nc.scalar.activation(r2[:, j * C:(j + 1) * C],
                                         l2[:, j * C:(j + 1) * C],
                                         COPY, scale=g2[:, j:j + 1])
                for j in (2 * qi, 2 * qi + 1):
                    for ri, rr in ((ridx[0], r1), (ridx[1], r2)):
                        nc.gpsimd.indirect_dma_start(
                            out=cw[:],
                            out_offset=bass.IndirectOffsetOnAxis(ap=ri[:, j:j + 1], axis=0),
                            in_=rr[:, j * C:(j + 1) * C],
                            in_offset=None,
                        )

            # ---- l_aux partials (off critical path) ----
            et = pool.tile([P, J * E], F32)
            sume = pool.tile([P, J], F32)
            for j in range(J):
                nc.scalar.activation(et[:, j * E:(j + 1) * E],
                                     lt[:, j * E:(j + 1) * E], EXP,
                                     accum_out=sume[:, j:j + 1])
            rcp = pool.tile([P, J], F32)
            nc.vector.reciprocal(rcp[:], sume[:])
            ones = pool.tile([P, 1], F32)
            nc.vector.memset(ones[:], 1.0)
            me_ps = psum_pool.tile([1, E], F32, space="PSUM")
            for j in range(J):
                nc.tensor.matmul(me_ps[:], lhsT=rcp[:, j:j + 1],
                                 rhs=et[:, j * E:(j + 1) * E],
                                 start=(j == 0), stop=(j == J - 1))
            ce_ps = psum_pool.tile([1, E], F32, space="PSUM")
            for j in range(J):
                nc.tensor.matmul(ce_ps[:], lhsT=ones[:],
                                 rhs=m1[:, j * E:(j + 1) * E],
                                 start=(j == 0), stop=(j == J - 1))
            part_sb = pool.tile([1, 2 * E], F32)
            nc.vector.tensor_copy(part_sb[:1, :E], me_ps[:])
            nc.vector.tensor_copy(part_sb[:1, E:], ce_ps[:])
            nc.sync.dma_start(partials[:], part_sb[:])
    nc.finalize()
    return nc


def _get_nc():
    if "nc" not in _CACHE:
        _CACHE["nc"] = _build()
    return _CACHE["nc"]


def _in_maps(logits, mask1_float, mask2_float, locations1_sc, locations2_sc):
    maps = []
    for c in range(N_CORES):
        sl = slice(c * S_LOC, (c + 1) * S_LOC)
        maps.append({
            "logits": np.ascontiguousarray(logits[sl]),
            "mask1": np.ascontiguousarray(mask1_float[sl]),
            "mask2": np.ascontiguousarray(mask2_float[sl]),
            "loc1": np.ascontiguousarray(locations1_sc[sl]),
            "loc2": np.ascontiguousarray(locations2_sc[sl]),
        })
    return maps


def _install_ntff_shim():
    """The agent image's antenv lacks axon_hooks; provide it so trace=True
    can capture NTFF profiles via the libaxon ctypes path."""
    import sys
    import types

    if "antenv.axon_hooks" in sys.modules:
        return
    try:
        import antenv
        from trn_agent_boot.trn_boot import _ntff_profile_via_ctypes

        mod = types.ModuleType("antenv.axon_hooks")
        hook = _ntff_profile_via_ctypes("/opt/axon/libaxon_pjrt.so")
        mod._hook = hook
        mod.set_axon_ntff_profile_hook = lambda h: setattr(mod, "_hook", h)
        mod.get_axon_ntff_profile_hook = lambda: mod._hook
        sys.modules["antenv.axon_hooks"] = mod
        antenv.axon_hooks = mod
    except Exception:
        pass


def _run(inputs, trace=False, **kwargs):
    if trace:
        _install_ntff_shim()
    nc = _get_nc()
    maps = _in_maps(**{k: np.asarray(v) for k, v in inputs.items()})
    return bass_utils.run_bass_kernel_spmd(
        nc, maps, core_ids=list(range(N_CORES)), trace=trace, **kwargs
    )


def _assemble(results):
    cw = np.concatenate(
        [results[c]["cw"].reshape(S_LOC, E, C) for c in range(N_CORES)], axis=0
    )
    me_sum = np.zeros(E, np.float64)
    ce_sum = np.zeros(E, np.float64)
    for c in range(N_CORES):
        part = results[c]["partials"].reshape(2 * E)
        me_sum += part[:E]
        ce_sum += part[E:]
    l_aux = np.float32(E * np.sum(me_sum * ce_sum) / (S * S))
    return l_aux, cw


def kernel(**inputs):
    res = _run(inputs)
    return _assemble(res.results)


# revision 7
# speedup vs baseline: 1.1236x; 1.1236x over previous
"""Trainium2 Bass kernel for MoE top-2 routing (softmax + l_aux + combine weights).

Problem: logits/mask1/mask2 [8192, 64], locations1/2 one-hot [8192, 256].
Outputs: l_aux scalar and combine_weights [8192, 64, 256].

Key structural facts exploited:
  * mask1/mask2 are disjoint one-hot rows and locations are one-hot rows, so
    combine_weights has exactly 2 nonzero elements per token out of 64*256.
    Each nonzero "row" combine_weights[s, e_k, :] equals g_k[s] * loc_k[s, :].
    We only *scatter* 2 rows of 256 floats per token into the (pre-zeroed)
    output via indirect DMA instead of materializing 512 MiB. The Q7
    descriptor generation (~18 ns/desc) dominates, so the whole rest of the
    kernel is arranged to hide behind the 16 scatter instructions.
  * g1 = num1/(num1+num2) with num_k = exp(top-k logit) simplifies to
    g1 = sigmoid(max - secondmax), g2 = sigmoid(secondmax - max); the
    softmax denominator cancels and the eps clamp never binds.
  * l_aux only needs column sums of gates (softmax) and mask1; both are
    computed as recip/ones-weighted PE matmuls accumulated in PSUM, entirely
    off the critical path; each core emits partial sums and the host
    finishes the tiny reduction while unsharding.

Sharding: tokens split 8 ways (1024 tokens per core); no cross-core
communication needed on device. Per-core token s = 8*p + j lives on
partition p, column group j (contiguous DRAM rows per partition => large
DMA descriptors on loads).
"""

import numpy as np

import concourse.bass as bass
import concourse.bacc as bacc
import concourse.mybir as mybir
from concourse.tile import TileContext
from concourse import bass_utils

S, E, C = 8192, 64, 256
N_CORES = 8
S_LOC = S // N_CORES          # 1024 tokens per core
P = 128                       # partitions
J = S_LOC // P                # 8 tokens per partition
F32 = mybir.dt.float32
AX = mybir.AxisListType.X
MUL = mybir.AluOpType.mult
ADD = mybir.AluOpType.add
SUB = mybir.AluOpType.subtract
EXP = mybir.ActivationFunctionType.Exp
SIG = mybir.ActivationFunctionType.Sigmoid
COPY = mybir.ActivationFunctionType.Copy

_CACHE = {}


def _build():
    nc = bacc.Bacc("TRN2", target_bir_lowering=False,
                   dynamic_dma_scratch_size=65536)

    logits = nc.declare_dram_parameter("logits", [S_LOC, E], F32, isOutput=False)
    m1d = nc.declare_dram_parameter("mask1", [S_LOC, E], F32, isOutput=False)
    m2d = nc.declare_dram_parameter("mask2", [S_LOC, E], F32, isOutput=False)
    l1d = nc.declare_dram_parameter("loc1", [S_LOC, C], F32, isOutput=False)
    l2d = nc.declare_dram_parameter("loc2", [S_LOC, C], F32, isOutput=False)
    cw = nc.declare_dram_parameter("cw", [S_LOC * E, C], F32, isOutput=True)
    partials = nc.declare_dram_parameter("partials", [1, 2 * E], F32, isOutput=True)

    with TileContext(nc) as tc:
        with (
            tc.tile_pool(name="sbuf", bufs=1) as pool,
            tc.tile_pool(name="psum", bufs=1, space="PSUM") as psum_pool,
        ):
            def v3(tile, inner):  # [P, J*inner] -> [P, J, inner]
                return tile[:].rearrange("p (j i) -> p j i", j=J)

            # ---- input loads (HWDGE, contiguous rows per partition) ----
            m1 = pool.tile([P, J * E], F32)
            nc.sync.dma_start(m1[:], m1d[:].rearrange("(p j) e -> p (j e)", p=P))
            lt = pool.tile([P, J * E], F32)
            nc.sync.dma_start(lt[:], logits[:].rearrange("(p j) e -> p (j e)", p=P))
            m2 = pool.tile([P, J * E], F32)
            nc.sync.dma_start(m2[:], m2d[:].rearrange("(p j) e -> p (j e)", p=P))
            l1 = pool.tile([P, J * C], F32)
            l2 = pool.tile([P, J * C], F32)
            l1v = l1d[:].rearrange("(p j) c -> p (j c)", p=P)
            l2v = l2d[:].rearrange("(p j) c -> p (j c)", p=P)
            H = J * C // 2  # half of the loc row block (4 tokens)
            nc.sync.dma_start(l1[:, :H], l1v[:, :H])
            nc.sync.dma_start(l2[:, :H], l2v[:, :H])
            nc.sync.dma_start(l1[:, H:], l1v[:, H:])
            nc.sync.dma_start(l2[:, H:], l2v[:, H:])

            # rowvals[p, j*E + e] = (8p+j)*E + e via iota (no DMA needed)
            rvi = pool.tile([P, J * E], mybir.dt.int32)
            nc.gpsimd.iota(rvi[:], pattern=[[E, J], [1, E]], base=0,
                           channel_multiplier=J * E)
            rv = pool.tile([P, J * E], F32)
            nc.vector.tensor_copy(rv[:], rvi[:])

            # ---- scatter row indices (critical: term1 first) ----
            ridx = []
            for k, mk in enumerate((m1, m2)):
                q = pool.tile([P, J * E], F32, tag=f"q{k}")
                nc.vector.tensor_tensor(v3(q, E), v3(mk, E), v3(rv, E), op=MUL)
                rf = pool.tile([P, J], F32, tag=f"rf{k}")
                nc.vector.reduce_sum(rf[:], v3(q, E), axis=AX)
                ri = pool.tile([P, J], mybir.dt.int32, tag=f"ri{k}")
                nc.vector.tensor_copy(ri[:], rf[:])
                ridx.append(ri)

            # ---- gate values: g1 = sigmoid(max - secondmax), g2 = 1 - g1 ----
            rmax = pool.tile([P, J], F32)
            nc.vector.reduce_max(rmax[:], v3(lt, E), axis=AX)
            p2 = pool.tile([P, J * E], F32)
            nc.vector.tensor_tensor(v3(p2, E), v3(m2, E), v3(lt, E), op=MUL)
            sm2 = pool.tile([P, J], F32)  # secondmax (mask2 selects it; rest<=0 adds 0... not true, logits can be negative)
            nc.vector.reduce_sum(sm2[:], v3(p2, E), axis=AX)
            dlt = pool.tile([P, J], F32)
            nc.vector.tensor_tensor(dlt[:], rmax[:], sm2[:], op=SUB)
            g1 = pool.tile([P, J], F32)
            nc.scalar.activation(g1[:], dlt[:], SIG)
            ndlt = pool.tile([P, J], F32)
            nc.vector.tensor_scalar_mul(ndlt[:], dlt[:], -1.0)
            g2 = pool.tile([P, J], F32)
            nc.scalar.activation(g2[:], ndlt[:], SIG)

            # ---- payload rows (all DVE/ACT work before the scatter window,
            # so DVE 2-port traffic never locks GpSimd out of its rings) ----
            r1 = pool.tile([P, J * C], F32)
            r2 = pool.tile([P, J * C], F32)
            g1b = g1[:].broadcast_to([P, J, C])
            for hi in range(2):
                js = slice(4 * hi, 4 * hi + 4)
                nc.vector.tensor_tensor(v3(r1, C)[:, js], v3(l1, C)[:, js],
                                        g1b[:, js], op=MUL)
                for j in range(4 * hi, 4 * hi + 4):
                    nc.scalar.activation(r2[:, j * C:(j + 1) * C],
                                         l2[:, j * C:(j + 1) * C],
                                         COPY, scale=g2[:, j:j + 1])

            # ---- l_aux softmax pieces (ACT/DVE, before the window) ----
            et = pool.tile([P, J * E], F32)
            sume = pool.tile([P, J], F32)
            for j in range(J):
                nc.scalar.activation(et[:, j * E:(j + 1) * E],
                                     lt[:, j * E:(j + 1) * E], EXP,
                                     accum_out=sume[:, j:j + 1])
            rcp = pool.tile([P, J], F32)
            nc.vector.reciprocal(rcp[:], sume[:])
            ones = pool.tile([P, 1], F32)
            nc.vector.memset(ones[:], 1.0)

            # ---- the scatter window: 16 back-to-back indirect DMAs;
            # term-1 rows first (term-2 payload finishes while they run) ----
            for ri, rr in ((ridx[0], r1), (ridx[1], r2)):
                for j in range(J):
                    nc.gpsimd.indirect_dma_start(
                        out=cw[:],
                        out_offset=bass.IndirectOffsetOnAxis(ap=ri[:, j:j + 1], axis=0),
                        in_=rr[:, j * C:(j + 1) * C],
                        in_offset=None,
                    )

            # ---- l_aux partials via PE (PSUM accumulate; overlaps window) ----
            me_ps = psum_pool.tile([1, E], F32, space="PSUM")
            for j in range(J):
                nc.tensor.matmul(me_ps[:], lhsT=rcp[:, j:j + 1],
                                 rhs=et[:, j * E:(j + 1) * E],
                                 start=(j == 0), stop=(j == J - 1))
            ce_ps = psum_pool.tile([1, E], F32, space="PSUM")
            for j in range(J):
                nc.tensor.matmul(ce_ps[:], lhsT=ones[:],
                                 rhs=m1[:, j * E:(j + 1) * E],
                                 start=(j == 0), stop=(j == J - 1))
            part_sb = pool.tile([1, 2 * E], F32)
            nc.vector.tensor_copy(part_sb[:1, :E], me_ps[:])
            nc.vector.tensor_copy(part_sb[:1, E:], ce_ps[:])
            nc.sync.dma_start(partials[:], part_sb[:])
    nc.finalize()
    return nc


def _get_nc():
    if "nc" not in _CACHE:
        _CACHE["nc"] = _build()
    return _CACHE["nc"]


def _in_maps(logits, mask1_float, mask2_float, locations1_sc, locations2_sc):
    maps = []
    for c in range(N_CORES):
        sl = slice(c * S_LOC, (c + 1) * S_LOC)
        maps.append({
            "logits": np.ascontiguousarray(logits[sl]),
            "mask1": np.ascontiguousarray(mask1_float[sl]),
            "mask2": np.ascontiguousarray(mask2_float[sl]),
            "loc1": np.ascontiguousarray(locations1_sc[sl]),
            "loc2": np.ascontiguousarray(locations2_sc[sl]),
        })
    return maps


def _install_ntff_shim():
    """The agent image's antenv lacks axon_hooks; provide it so trace=True
    can capture NTFF profiles via the libaxon ctypes path."""
    import sys
    import types

    if "antenv.axon_hooks" in sys.modules:
        return
    try:
        import antenv
        from trn_agent_boot.trn_boot import _ntff_profile_via_ctypes

        mod = types.ModuleType("antenv.axon_hooks")
        hook = _ntff_profile_via_ctypes("/opt/axon/libaxon_pjrt.so")
        mod._hook = hook
        mod.set_axon_ntff_profile_hook = lambda h: setattr(mod, "_hook", h)
        mod.get_axon_ntff_profile_hook = lambda: mod._hook
        sys.modules["antenv.axon_hooks"] = mod
        antenv.axon_hooks = mod
    except Exception:
        pass


def _run(inputs, trace=False, **kwargs):
    if trace:
        _install_ntff_shim()
    nc = _get_nc()
    maps = _in_maps(**{k: np.asarray(v) for k, v in inputs.items()})
    return bass_utils.run_bass_kernel_spmd(
        nc, maps, core_ids=list(range(N_CORES)), trace=trace, **kwargs
    )


def _assemble(results):
    cw = np.concatenate(
        [results[c]["cw"].reshape(S_LOC, E, C) for c in range(N_CORES)], axis=0
    )
    me_sum = np.zeros(E, np.float64)
    ce_sum = np.zeros(E, np.float64)
    for c in range(N_CORES):
        part = results[c]["partials"].reshape(2 * E)
        me_sum += part[:E]
        ce_sum += part[E:]
    l_aux = np.float32(E * np.sum(me_sum * ce_sum) / (S * S))
    return l_aux, cw


def kernel(**inputs):
    res = _run(inputs)
    return _assemble(res.results)
